# revision 1
# baseline (speedup 1.0000x reference)
"""3D Haar DWT (depth-1) Trainium2 kernel.

Full inputs: x [4, 4, 64, 256, 256] f32 + six banded Haar matrices
(hardcoded math: every output element is +-2^-1.5 times a +-sum of a
2x2x2 block). Returns the 8 subbands (LLL, LLH, LHL, LHH, HLL, HLH,
HHL, HHH), each [4, 4, 32, 128, 128] f32.

Sharding: data-parallel over N*C = 16 sample-channels, 2 per core on
8 cores. Per-core compute is a 3-stage butterfly over pair-packed
tiles (SBUF partition p holds input rows 2p and 2p+1 contiguously, so
every DMA descriptor is a 2 KiB linear run):
  H stage: row pairs    -> TensorE matmuls against +-2^-1.5 * I for
                           3 of every 4 d-pairs (fp32, exact);
                           DVE adds + ScalarE pre-scale for the 4th.
  W stage: column pairs -> DVE stride-2 tensor_add/sub (FD=1024)
  D stage: slice pairs  -> DVE tensor_add/sub (FD=1024, 4-D APs
                           covering two subbands per instruction)
ScalarE evacuates PSUM. Everything stays fp32-exact.
"""
import sys

sys.path.insert(0, "/opt/trn_rl_repo")

import numpy as np

N, C, D, H, W = 4, 4, 64, 256, 256
NCORES = 8
G_PER_CORE = (N * C) // NCORES        # 2
KP = D // 2                           # 32 d-pairs per g
S3 = float(2.0 ** -1.5)

# schedule tunables
KB = 8                                # k-slices per output staging block
IN_BUFS = 8
EV_BUFS = 4
WT_BUFS = 2
OS_BUFS = 2
PSUM_BUFS = 3

_CACHE = {}


def _build_filter_lhst():
    """Stationary operands: +S3*I and -S3*I, as [2, 128, 128] fp32."""
    eye = np.eye(128, dtype=np.float32)
    return np.stack([np.float32(S3) * eye, np.float32(-S3) * eye])


def _build_nc():
    import concourse.bass as bass
    import concourse.tile as tile
    from concourse import bacc, mybir

    f32 = mybir.dt.float32
    nc = bacc.Bacc(None)
    x_d = nc.declare_dram_parameter("x", [G_PER_CORE, D, H, W], f32,
                                    isOutput=False)
    ft_d = nc.declare_dram_parameter("ft", [2, 128, 128], f32,
                                     isOutput=False)
    # h'-major layout: per (s, g, partition=h') a k-block of 8 is one
    # contiguous 4 KiB run in DRAM (host transposes k and h' back)
    o_d = nc.declare_dram_parameter("out", [8, G_PER_CORE, 128, KP, 128],
                                    f32, isOutput=True)

    with tile.TileContext(nc) as tc:
        with (
            tc.tile_pool(name="cst", bufs=1) as cst,
            tc.tile_pool(name="inp", bufs=IN_BUFS) as inp,
            tc.tile_pool(name="ev", bufs=EV_BUFS) as evp,
            tc.tile_pool(name="wt", bufs=WT_BUFS) as wtp,
            tc.tile_pool(name="os", bufs=OS_BUFS) as osp,
            tc.tile_pool(name="ps", bufs=PSUM_BUFS, space="PSUM") as psp,
        ):
            ft = cst.tile([128, 256], f32, tag="ft")
            nc.sync.dma_start(
                ft.rearrange("p (i c) -> p i c", i=2),
                ft_d.rearrange("i p c -> p i c"))
            pos_i = ft[:, 0:128]    # +S3 * I
            neg_i = ft[:, 128:256]  # -S3 * I

            def load_pair(g, k):
                """One d-pair as a pair-packed tile [128, 1024]:
                cols = {s0: row2p row2p+1 | s1: row2p row2p+1}."""
                t = inp.tile([128, 1024], f32, tag="xin")
                nc.sync.dma_start(
                    t.rearrange("p (s r) -> p s r", s=2),
                    x_d[g, 2 * k:2 * k + 2].rearrange(
                        "s (p r) w -> p s (r w)", r=2))
                return t

            for g in range(G_PER_CORE):
                for kb in range(KP // KB):
                    os_t = osp.tile([128, 8 * KB * 128], f32, tag="os")
                    for half in range(KB // 4):
                        wt_t = wtp.tile([128, 4 * 1024], f32, tag="wt")
                        # EV tiles: j0+j1 (both PE), j2 (PE) + j3 (DVE)
                        ev01 = evp.tile([128, 2048], f32, tag="ev")
                        ev23 = evp.tile([128, 2048], f32, tag="ev")
                        for j in range(4):
                            k = kb * KB + half * 4 + j
                            t = load_pair(g, k)
                            t4 = t.rearrange("p (s r w) -> p s r w",
                                             s=2, r=2)
                            if j < 3:
                                # --- H stage on TensorE: +-S3*I matmuls
                                pt = psp.tile([128, 1024], f32, tag="ps")
                                lo = pt[:, 0:512].rearrange(
                                    "p (s w) -> p s w", s=2)
                                hi = pt[:, 512:1024].rearrange(
                                    "p (s w) -> p s w", s=2)
                                nc.tensor.matmul(lo, pos_i, t4[:, :, 0, :],
                                                 start=True, stop=False)
                                nc.tensor.matmul(lo, pos_i, t4[:, :, 1, :],
                                                 start=False, stop=True)
                                nc.tensor.matmul(hi, pos_i, t4[:, :, 0, :],
                                                 start=True, stop=False)
                                nc.tensor.matmul(hi, neg_i, t4[:, :, 1, :],
                                                 start=False, stop=True)
                                # ScalarE evacuation (scaled via weights)
                                dst = (ev01[:, j * 1024:(j + 1) * 1024]
                                       if j < 2 else ev23[:, 0:1024])
                                nc.scalar.activation(
                                    dst, pt[:],
                                    mybir.ActivationFunctionType.Copy)
                            else:
                                # --- H stage on DVE (ScalarE pre-scale)
                                nc.scalar.activation(
                                    t[:], t[:],
                                    mybir.ActivationFunctionType.Copy,
                                    bias=0.0, scale=S3)
                                pl = ev23[:, 1024:2048]
                                pl3 = pl.rearrange("p (b s w) -> p b s w",
                                                   b=2, s=2)
                                nc.vector.tensor_add(
                                    pl3[:, 0], t4[:, :, 0, :],
                                    t4[:, :, 1, :])
                                nc.vector.tensor_sub(
                                    pl3[:, 1], t4[:, :, 0, :],
                                    t4[:, :, 1, :])
                        # --- W stage on DVE, FD=1024 ---
                        # ev layout per 1024: {A_lo(s0,s1) | A_hi(s0,s1)}
                        wt4 = wt_t.rearrange("p (j b) -> p j b", j=4)
                        for ev, j0 in ((ev01, 0), (ev23, 2)):
                            nc.vector.tensor_add(
                                wt4[:, j0:j0 + 2, 0:512],
                                ev[:, 0::2].rearrange(
                                    "p (j b) -> p j b", j=2),
                                ev[:, 1::2].rearrange(
                                    "p (j b) -> p j b", j=2))
                            nc.vector.tensor_sub(
                                wt4[:, j0:j0 + 2, 512:1024],
                                ev[:, 0::2].rearrange(
                                    "p (j b) -> p j b", j=2),
                                ev[:, 1::2].rearrange(
                                    "p (j b) -> p j b", j=2))
                        # --- D stage, FD=1024, two subbands per op ---
                        # wt_t per-pair block (j): {LL0 LL1 HL0 HL1 |
                        #                           LH0 LH1 HH0 HH1}
                        wtd = wt_t.rearrange("p (j c w) -> p c j w",
                                             j=4, c=8)
                        osd = os_t.rearrange("p (s q w) -> p s q w",
                                             s=8, q=KB)
                        qs = slice(half * 4, half * 4 + 4)
                        for c0, s_sum, s_diff in ((0, 0, 4), (4, 1, 5)):
                            # c blocks {c0, c0+2} = {LL,HL} / {LH,HH}
                            in0 = wtd[:, c0:c0 + 3:2]
                            in1 = wtd[:, c0 + 1:c0 + 4:2]
                            nc.vector.tensor_add(
                                osd[:, s_sum:s_sum + 3:2, qs], in0, in1)
                            nc.vector.tensor_sub(
                                osd[:, s_diff:s_diff + 3:2, qs], in0, in1)
                    # --- store this k-block: 8 subbands x [128,KB,128] ---
                    for s in range(8):
                        src_ap = os_t[:, s * KB * 128:(s + 1) * KB * 128]
                        nc.sync.dma_start(
                            o_d[s, g, :, kb * KB:(kb + 1) * KB, :],
                            src_ap.rearrange("p (q w) -> p q w", q=KB))
    nc.finalize()
    return nc


def _get_nc():
    if "nc" not in _CACHE:
        _CACHE["nc"] = _build_nc()
    return _CACHE["nc"]


def kernel(x, low_0, low_1, low_2, high_0, high_1, high_2):
    from concourse.bass_utils import run_bass_kernel_spmd

    x = np.ascontiguousarray(np.asarray(x, dtype=np.float32))
    ft = _build_filter_lhst()
    xs = x.reshape(N * C, D, H, W)
    in_maps = [
        {"x": np.ascontiguousarray(
            xs[c * G_PER_CORE:(c + 1) * G_PER_CORE]), "ft": ft}
        for c in range(NCORES)
    ]
    nc = _get_nc()
    res = run_bass_kernel_spmd(nc, in_maps, list(range(NCORES)))
    full = np.empty((8, N * C, KP, 128, 128), dtype=np.float32)
    for c in range(NCORES):
        full[:, c * G_PER_CORE:(c + 1) * G_PER_CORE] = \
            res.results[c]["out"].transpose(0, 1, 3, 2, 4)
    full = full.reshape(8, N, C, KP, 128, 128)
    return tuple(full[s] for s in range(8))



# revision 2
# speedup vs baseline: 2.3943x; 2.3943x over previous
"""3D Haar DWT (depth-1) Trainium2 kernel — bf16 pipeline.

Full inputs: x [4, 4, 64, 256, 256] f32 + six banded Haar matrices
(hardcoded math: every output element is +-2^-1.5 times a +-sum of a
2x2x2 block). Returns the 8 subbands (LLL, LLH, LHL, LHH, HLL, HLH,
HHL, HHH), each [4, 4, 32, 128, 128] f32.

Tolerance is 2e-2 max-abs-relative, so the whole pipeline runs in
bf16 on device (input cast + output cast happen on host): DMA traffic
halves to 33.6 MB/core and DVE tensor ops hit the 2x_1P perf mode.

Sharding: data-parallel over N*C = 16 sample-channels, 2 per core on
8 cores. Per-core compute is a 3-stage butterfly over pair-packed
tiles (SBUF partition p holds input rows 2p and 2p+1 contiguously, so
every load descriptor is a 1 KiB linear run):
  H stage: row pairs    -> TensorE matmuls against +-2^-1.5 * I; the
                           moving APs split w into (even, odd) halves
                           so PSUM comes out parity-deinterleaved.
  evac:    ScalarE copies PSUM (fp32) -> SBUF (bf16).
  W stage: parity halves-> DVE tensor_add/sub, unit-stride, 2x mode
  D stage: slice pairs  -> DVE tensor_add/sub, unit-stride, 2x mode
Stores stage KB d-pairs so each DMA writes one 16 KiB run/partition.
"""
import sys

sys.path.insert(0, "/opt/trn_rl_repo")

import numpy as np
import ml_dtypes

N, C, D, H, W = 4, 4, 64, 256, 256
NCORES = 8
G_PER_CORE = (N * C) // NCORES        # 2
KP = D // 2                           # 32 d-pairs per g
S3 = float(2.0 ** -1.5)
BF16 = ml_dtypes.bfloat16

# schedule tunables
KB = 8                                # d-pairs per load/store block
IN_BUFS = 3
EV_BUFS = 8
WT_BUFS = 8
OS_BUFS = 2
PSUM_BUFS = 4

_CACHE = {}


def _build_filter_lhst():
    """Stationary operands: +S3*I and -S3*I, as [2, 128, 128] bf16."""
    eye = np.eye(128, dtype=np.float32)
    return np.stack([np.float32(S3) * eye,
                     np.float32(-S3) * eye]).astype(BF16)


def _build_nc():
    import concourse.tile as tile
    from concourse import bacc, mybir

    f32 = mybir.dt.float32
    bf16 = mybir.dt.bfloat16
    NB = KP // KB                     # store blocks per g
    nc = bacc.Bacc(None)
    x_d = nc.declare_dram_parameter("x", [G_PER_CORE, D, H, W], bf16,
                                    isOutput=False)
    ft_d = nc.declare_dram_parameter("ft", [2, 128, 128], bf16,
                                     isOutput=False)
    # per (g, partition=h', kb): one contiguous 16 KiB run holding
    # [band, kq, u] (host transposes back to subband-major)
    o_d = nc.declare_dram_parameter("out", [G_PER_CORE, 128, NB, 8, KB, 128],
                                    bf16, isOutput=True)

    with tile.TileContext(nc) as tc:
        with (
            tc.tile_pool(name="cst", bufs=1) as cst,
            tc.tile_pool(name="inp", bufs=IN_BUFS) as inp,
            tc.tile_pool(name="ev", bufs=EV_BUFS) as evp,
            tc.tile_pool(name="wt", bufs=WT_BUFS) as wtp,
            tc.tile_pool(name="os", bufs=OS_BUFS) as osp,
            tc.tile_pool(name="ps", bufs=PSUM_BUFS, space="PSUM") as psp,
        ):
            ft = cst.tile([128, 256], bf16, tag="ft")
            nc.sync.dma_start(
                ft.rearrange("p (i c) -> p i c", i=2),
                ft_d.rearrange("i p c -> p i c"))
            pos_i = ft[:, 0:128]    # +S3 * I
            neg_i = ft[:, 128:256]  # -S3 * I

            for g in range(G_PER_CORE):
                for kb in range(NB):
                    # one 2 MiB load: KB d-pairs, pair-packed
                    blk = inp.tile([128, KB * 1024], bf16, tag="xin")
                    nc.sync.dma_start(
                        blk.rearrange("p (k s r) -> p k s r", k=KB, s=2),
                        x_d[g, kb * 2 * KB:(kb + 1) * 2 * KB].rearrange(
                            "(k s) (p r) w -> p k s (r w)", s=2, r=2))
                    os_t = osp.tile([128, 8 * KB * 128], bf16, tag="os")
                    osd = os_t.rearrange("p (dd b q kq u) -> p dd b q kq u",
                                         dd=2, b=2, q=2, kq=KB)
                    for kq in range(KB):
                        tk = blk[:, kq * 1024:(kq + 1) * 1024]
                        # w = 2u + par; moving APs iterate (s, par, u)
                        mov = tk.rearrange("p (s r u par) -> p r s par u",
                                           s=2, r=2, par=2)
                        # --- H stage on TensorE: psum [b, s, par, u] ---
                        pt = psp.tile([128, 1024], f32, tag="ps")
                        pt3 = pt.rearrange("p (b s par u) -> p b s par u",
                                           b=2, s=2, par=2)
                        nc.tensor.matmul(pt3[:, 0], pos_i, mov[:, 0],
                                         start=True, stop=False)
                        nc.tensor.matmul(pt3[:, 0], pos_i, mov[:, 1],
                                         start=False, stop=True)
                        nc.tensor.matmul(pt3[:, 1], pos_i, mov[:, 0],
                                         start=True, stop=False)
                        nc.tensor.matmul(pt3[:, 1], neg_i, mov[:, 1],
                                         start=False, stop=True)
                        # --- ScalarE evacuation: fp32 -> bf16 ---
                        ev = evp.tile([128, 1024], bf16, tag="ev")
                        nc.scalar.activation(
                            ev[:], pt[:], mybir.ActivationFunctionType.Copy)
                        ev4 = ev.rearrange("p (b s par u) -> p b s par u",
                                           b=2, s=2, par=2)
                        # --- W stage on DVE (unit stride, 2x) ---
                        wt_t = wtp.tile([128, 1024], bf16, tag="wt")
                        wt4 = wt_t.rearrange("p (b s q u) -> p b s q u",
                                             b=2, s=2, q=2)
                        nc.vector.tensor_add(wt4[:, :, :, 0],
                                             ev4[:, :, :, 0],
                                             ev4[:, :, :, 1])
                        nc.vector.tensor_sub(wt4[:, :, :, 1],
                                             ev4[:, :, :, 0],
                                             ev4[:, :, :, 1])
                        # --- D stage on DVE (unit stride, 2x) ---
                        nc.vector.tensor_add(osd[:, 0, :, :, kq],
                                             wt4[:, :, 0], wt4[:, :, 1])
                        nc.vector.tensor_sub(osd[:, 1, :, :, kq],
                                             wt4[:, :, 0], wt4[:, :, 1])
                    # one 2 MiB store: 16 KiB contiguous per partition
                    nc.sync.dma_start(
                        o_d[g, :, kb],
                        os_t.rearrange("p (band kq u) -> p band kq u",
                                       band=8, kq=KB))
    nc.finalize()
    return nc


def _get_nc():
    if "nc" not in _CACHE:
        _CACHE["nc"] = _build_nc()
    return _CACHE["nc"]


def _make_in_maps(x):
    xs = np.asarray(x, dtype=np.float32).reshape(N * C, D, H, W).astype(BF16)
    ft = _build_filter_lhst()
    return [
        {"x": np.ascontiguousarray(
            xs[c * G_PER_CORE:(c + 1) * G_PER_CORE]), "ft": ft}
        for c in range(NCORES)
    ]


def _unshard(core_outs):
    """core_outs[c]: [G, 128, NB, 8, KB, 128] bf16 -> 8 full f32 bands."""
    full = np.empty((8, N * C, KP, 128, 128), dtype=np.float32)
    for c, arr in enumerate(core_outs):
        a = np.asarray(arr).astype(np.float32)
        a = a.transpose(3, 0, 2, 4, 1, 5).reshape(8, G_PER_CORE, KP, 128, 128)
        full[:, c * G_PER_CORE:(c + 1) * G_PER_CORE] = a
    full = full.reshape(8, N, C, KP, 128, 128)
    return tuple(full[s] for s in range(8))


def kernel(x, low_0, low_1, low_2, high_0, high_1, high_2):
    from concourse.bass_utils import run_bass_kernel_spmd

    in_maps = _make_in_maps(x)
    nc = _get_nc()
    res = run_bass_kernel_spmd(nc, in_maps, list(range(NCORES)))
    return _unshard([res.results[c]["out"] for c in range(NCORES)])
